# revision 3
# baseline (speedup 1.0000x reference)
"""Data-parallel Trainium2 kernel for AdaptiveSignatureHedger.

Strategy (per sharding hint): pure data parallel. The batch B=32 is split
across the 8 NeuronCores (4 samples each); all parameters (a few hundred KB)
are replicated. There is no cross-sample communication, so each core runs the
full per-sample pipeline on its shard and the host concatenates the shards.

The per-shard forward is expressed in JAX and compiled with pmap to a NEFF
that runs on all 8 cores concurrently.
"""

import numpy as np
import jax
import jax.numpy as jnp

W = 10
DELTA_MAX = 1.5
BN_EPS = 1e-5

_B, _T, _D, _H = 32, 2048, 5, 64
_NCORES = 8


def _revcumsum(x, axis):
    return jnp.flip(jnp.cumsum(jnp.flip(x, axis), axis=axis), axis)


def _forward_shard(features, hw1, hb1, hw2, hb2, hw3, hb3, gw1, gb1, gw2, gb2,
                   p1, p1b, p2, p2b, p3, p3b, p4, p4b,
                   w1, b1, gamma, beta, rmean, rvar, w2, b2, w3, b3):
    B, T, d = features.shape
    f32 = features.dtype
    lp = features[:, :, 0]
    ret = jnp.pad(lp[:, 1:] - lp[:, :-1], ((0, 0), (1, 0)))

    # Hurst estimator over sliding windows of the return series.
    # Built from shifted slices (gathers compile poorly on neuronx).
    rw = jnp.stack([ret[:, w:T - W + w] for w in range(W)], axis=-1)
    hh = jax.nn.relu(rw @ hw1 + hb1)
    hh = jax.nn.relu(hh @ hw2 + hb2)
    H_tail = 0.5 * jax.nn.sigmoid((hh @ hw3 + hb3)[..., 0])    # [B, T-W]
    H_local = jnp.concatenate(
        [jnp.repeat(H_tail[:, :1], W, axis=1), H_tail], axis=1)
    vol = jnp.cumsum(jnp.abs(ret), axis=1) / (
        jnp.arange(1, T + 1, dtype=f32) + 1e-8)

    regime = jnp.stack([H_local, vol], axis=-1)                # [B, T, 2]
    gate = jax.nn.softmax(
        jax.nn.relu(regime @ gw1 + gb1) @ gw2 + gb2, axis=-1)  # [B, T, 4]

    # Windowed path increments; window at t covers increments t-W .. t-1.
    Dinc = features[:, 1:] - features[:, :-1]                  # [B, T-1, d]
    Dp = jnp.pad(Dinc, ((0, 0), (W, 0), (0, 0)))
    Dw = jnp.stack([Dp[:, w:w + T] for w in range(W)], axis=2)  # [B, T, W, d]
    S = _revcumsum(Dw, 2)
    lvl1 = S[:, :, 0]                                          # [B, T, d]
    A = Dw[..., :, None] * S[..., None, :]                     # [B, T, W, d, d]
    l2f = jnp.sum(A, axis=2).reshape(B, T, d * d)
    RA = _revcumsum(A, 2) - A
    l3f = jnp.einsum('btrd,btref->btdef', Dw, RA).reshape(B, T, d ** 3)
    l4f = (l2f[..., :, None] * l2f[..., None, :]).reshape(B, T, d ** 4)

    proj1 = lvl1 @ p1 + p1b
    proj2 = jnp.concatenate([lvl1, l2f], -1) @ p2 + p2b
    proj3 = jnp.concatenate([lvl1, l2f, l3f], -1) @ p3 + p3b
    proj4 = jnp.concatenate([lvl1, l2f, l3f, l4f], -1) @ p4 + p4b

    tv = jnp.arange(T)
    m2 = (tv >= 1).astype(f32)[None, :, None]
    m3 = (tv >= 2).astype(f32)[None, :, None]
    m4 = (tv >= 3).astype(f32)[None, :, None]
    sig_repr = (gate[..., 0:1] * proj1 + m2 * gate[..., 1:2] * proj2
                + m3 * gate[..., 2:3] * proj3 + m4 * gate[..., 3:4] * proj4)

    combined = jnp.concatenate([sig_repr, features], axis=-1)
    h1 = jax.nn.relu(combined @ w1 + b1)
    bn = gamma * (h1 - rmean) * jax.lax.rsqrt(rvar + BN_EPS) + beta
    h2 = jax.nn.relu(bn @ w2 + b2)
    return DELTA_MAX * jnp.tanh((h2 @ w3 + b3)[..., 0])        # [B, T]


_PARAM_NAMES = [
    'hw1', 'hb1', 'hw2', 'hb2', 'hw3', 'hb3', 'gw1', 'gb1', 'gw2', 'gb2',
    'p1', 'p1b', 'p2', 'p2b', 'p3', 'p3b', 'p4', 'p4b',
    'w1', 'b1', 'gamma', 'beta', 'rmean', 'rvar', 'w2', 'b2', 'w3', 'b3',
]

_pmapped = None


def _get_pmapped():
    global _pmapped
    if _pmapped is None:
        # Batch axis 0 is mapped over the 8 cores; params are replicated.
        _pmapped = jax.pmap(
            _forward_shard,
            axis_name='cores',
            in_axes=(0,) + (None,) * len(_PARAM_NAMES),
            devices=jax.devices()[:_NCORES],
        )
    return _pmapped


def kernel(**inputs: np.ndarray) -> np.ndarray:
    features = np.asarray(inputs['features'], dtype=np.float32)
    B, T, d = features.shape
    assert B % _NCORES == 0, (B, _NCORES)
    bl = B // _NCORES

    # Shard batch across the 8 NeuronCores.
    shards = features.reshape(_NCORES, bl, T, d)
    params = [np.asarray(inputs[n], dtype=np.float32) for n in _PARAM_NAMES]

    fn = _get_pmapped()
    out = fn(shards, *params)                    # [8, bl, T]
    out = np.asarray(jax.device_get(out), dtype=np.float32)
    return out.reshape(B, T)


if __name__ == '__main__':
    ins = {
        'features': np.random.randn(_B, _T, _D).astype(np.float32) * 0.1,
    }
    rng = np.random.default_rng(0)
    shapes = dict(hw1=(10, 32), hb1=(32,), hw2=(32, 32), hb2=(32,),
                  hw3=(32, 1), hb3=(1,), gw1=(2, 32), gb1=(32,),
                  gw2=(32, 4), gb2=(4,), p1=(5, 64), p1b=(64,),
                  p2=(30, 64), p2b=(64,), p3=(155, 64), p3b=(64,),
                  p4=(780, 64), p4b=(64,), w1=(69, 64), b1=(64,),
                  gamma=(64,), beta=(64,), rmean=(64,), rvar=(64,),
                  w2=(64, 32), b2=(32,), w3=(32, 1), b3=(1,))
    for k, s in shapes.items():
        ins[k] = (rng.standard_normal(s) * 0.1).astype(np.float32)
    ins['rvar'] = np.abs(ins['rvar']) + 1.0
    o = kernel(**ins)
    print('kernel output', o.shape, o.dtype, float(np.abs(o).max()))


# revision 4
# speedup vs baseline: 1.0362x; 1.0362x over previous
"""Data-parallel Trainium2 kernel for AdaptiveSignatureHedger.

Strategy (per sharding hint): pure data parallel. The batch B=32 is split
across the 8 NeuronCores (4 samples each); all parameters (a few hundred KB)
are replicated. There is no cross-sample communication, so each core runs the
full per-sample pipeline on its shard and the host concatenates the shards.

The per-shard forward is expressed in JAX and compiled with pmap to a NEFF
that runs on all 8 cores concurrently.
"""

import numpy as np
import jax
import jax.numpy as jnp

W = 10
DELTA_MAX = 1.5
BN_EPS = 1e-5

_B, _T, _D, _H = 32, 2048, 5, 64
_NCORES = 8


def _revcumsum(x, axis):
    return jnp.flip(jnp.cumsum(jnp.flip(x, axis), axis=axis), axis)


def _forward_shard(features, hw1, hb1, hw2, hb2, hw3, hb3, gw1, gb1, gw2, gb2,
                   p1, p1b, p2, p2b, p3, p3b, p4, p4b,
                   w1, b1, gamma, beta, rmean, rvar, w2, b2, w3, b3):
    B, T, d = features.shape
    f32 = features.dtype
    lp = features[:, :, 0]
    ret = jnp.pad(lp[:, 1:] - lp[:, :-1], ((0, 0), (1, 0)))

    # Hurst estimator over sliding windows of the return series.
    # Built from shifted slices (gathers compile poorly on neuronx).
    rw = jnp.stack([ret[:, w:T - W + w] for w in range(W)], axis=-1)
    hh = jax.nn.relu(rw @ hw1 + hb1)
    hh = jax.nn.relu(hh @ hw2 + hb2)
    H_tail = 0.5 * jax.nn.sigmoid((hh @ hw3 + hb3)[..., 0])    # [B, T-W]
    H_local = jnp.concatenate(
        [jnp.repeat(H_tail[:, :1], W, axis=1), H_tail], axis=1)
    vol = jnp.cumsum(jnp.abs(ret), axis=1) / (
        jnp.arange(1, T + 1, dtype=f32) + 1e-8)

    regime = jnp.stack([H_local, vol], axis=-1)                # [B, T, 2]
    gate = jax.nn.softmax(
        jax.nn.relu(regime @ gw1 + gb1) @ gw2 + gb2, axis=-1)  # [B, T, 4]

    # Windowed path increments; window at t covers increments t-W .. t-1.
    Dinc = features[:, 1:] - features[:, :-1]                  # [B, T-1, d]
    Dp = jnp.pad(Dinc, ((0, 0), (W, 0), (0, 0)))
    Dw = jnp.stack([Dp[:, w:w + T] for w in range(W)], axis=2)  # [B, T, W, d]
    S = _revcumsum(Dw, 2)
    lvl1 = S[:, :, 0]                                          # [B, T, d]
    A = Dw[..., :, None] * S[..., None, :]                     # [B, T, W, d, d]
    l2f = jnp.sum(A, axis=2).reshape(B, T, d * d)
    RA = _revcumsum(A, 2) - A                                  # [B, T, W, d, d]
    RAf = RA.reshape(B, T, W, d * d)
    # einsum over the W axis as an unrolled elementwise sum — avoids lowering
    # to per-token batched matmuls on the tensor engine.
    l3f = sum(Dw[:, :, r, :, None] * RAf[:, :, r, None, :] for r in range(W))
    l3f = l3f.reshape(B, T, d ** 3)
    l4f = (l2f[..., :, None] * l2f[..., None, :]).reshape(B, T, d ** 4)

    proj1 = lvl1 @ p1 + p1b
    proj2 = jnp.concatenate([lvl1, l2f], -1) @ p2 + p2b
    proj3 = jnp.concatenate([lvl1, l2f, l3f], -1) @ p3 + p3b
    proj4 = jnp.concatenate([lvl1, l2f, l3f, l4f], -1) @ p4 + p4b

    tv = jnp.arange(T)
    m2 = (tv >= 1).astype(f32)[None, :, None]
    m3 = (tv >= 2).astype(f32)[None, :, None]
    m4 = (tv >= 3).astype(f32)[None, :, None]
    sig_repr = (gate[..., 0:1] * proj1 + m2 * gate[..., 1:2] * proj2
                + m3 * gate[..., 2:3] * proj3 + m4 * gate[..., 3:4] * proj4)

    combined = jnp.concatenate([sig_repr, features], axis=-1)
    h1 = jax.nn.relu(combined @ w1 + b1)
    bn = gamma * (h1 - rmean) * jax.lax.rsqrt(rvar + BN_EPS) + beta
    h2 = jax.nn.relu(bn @ w2 + b2)
    return DELTA_MAX * jnp.tanh((h2 @ w3 + b3)[..., 0])        # [B, T]


_PARAM_NAMES = [
    'hw1', 'hb1', 'hw2', 'hb2', 'hw3', 'hb3', 'gw1', 'gb1', 'gw2', 'gb2',
    'p1', 'p1b', 'p2', 'p2b', 'p3', 'p3b', 'p4', 'p4b',
    'w1', 'b1', 'gamma', 'beta', 'rmean', 'rvar', 'w2', 'b2', 'w3', 'b3',
]

_pmapped = None


def _get_pmapped():
    global _pmapped
    if _pmapped is None:
        # Batch axis 0 is mapped over the 8 cores; params are replicated.
        _pmapped = jax.pmap(
            _forward_shard,
            axis_name='cores',
            in_axes=(0,) + (None,) * len(_PARAM_NAMES),
            devices=jax.devices()[:_NCORES],
        )
    return _pmapped


def kernel(**inputs: np.ndarray) -> np.ndarray:
    features = np.asarray(inputs['features'], dtype=np.float32)
    B, T, d = features.shape
    assert B % _NCORES == 0, (B, _NCORES)
    bl = B // _NCORES

    # Shard batch across the 8 NeuronCores.
    shards = features.reshape(_NCORES, bl, T, d)
    params = [np.asarray(inputs[n], dtype=np.float32) for n in _PARAM_NAMES]

    fn = _get_pmapped()
    out = fn(shards, *params)                    # [8, bl, T]
    out = np.asarray(jax.device_get(out), dtype=np.float32)
    return out.reshape(B, T)


if __name__ == '__main__':
    ins = {
        'features': np.random.randn(_B, _T, _D).astype(np.float32) * 0.1,
    }
    rng = np.random.default_rng(0)
    shapes = dict(hw1=(10, 32), hb1=(32,), hw2=(32, 32), hb2=(32,),
                  hw3=(32, 1), hb3=(1,), gw1=(2, 32), gb1=(32,),
                  gw2=(32, 4), gb2=(4,), p1=(5, 64), p1b=(64,),
                  p2=(30, 64), p2b=(64,), p3=(155, 64), p3b=(64,),
                  p4=(780, 64), p4b=(64,), w1=(69, 64), b1=(64,),
                  gamma=(64,), beta=(64,), rmean=(64,), rvar=(64,),
                  w2=(64, 32), b2=(32,), w3=(32, 1), b3=(1,))
    for k, s in shapes.items():
        ins[k] = (rng.standard_normal(s) * 0.1).astype(np.float32)
    ins['rvar'] = np.abs(ins['rvar']) + 1.0
    o = kernel(**ins)
    print('kernel output', o.shape, o.dtype, float(np.abs(o).max()))
